# revision 28
# baseline (speedup 1.0000x reference)
"""Trainium2 Bass kernel for GQA sliding-window attention (8-core SPMD).

Problem: B=8, S=32, D=4096, H=32 Q-heads, KVH=8 KV-heads, HD=128,
sliding window 4096 with 4064 cached positions.

Sharding: tensor-parallel over heads. Core c owns Q heads 4c..4c+3 and KV
head c (one GQA group): Wq/Wk/Wv column-sharded, cache sharded by KV head,
x replicated. Attention runs in two head-pair passes; each pass's (bf16)
attention output is AllGathered (the axon-tunneled cores reject raw
cross-core remote_dma, so the ncfw collective is the only path; its
~14us floor + mesh wire time dominate the tail). Each core then applies
a column slice of Wo; host concatenates slices.

Key structure vs a straightforward port:
  - The 256 new tokens are folded into the cached-KV chunk loop as two
    extra [128, *] chunks. Cross-batch (and would-be masked) score entries
    get a host-built additive bias of -30 (exp -> ~1e-13, vanishes in the
    4096-term softmax sums); the reference's additive mask is folded into
    the same bias tile. This removes the per-batch PSUM-serialized block
    the baseline had.
  - Softmax skips max-subtraction (scores are O(10), fp32 exp is safe);
    row sums are accumulated on the Vector engine (fp32) from the bf16
    exp tiles, then reduced across partitions with a single ones-matmul
    per pass; 1/sum is broadcast across partitions with a rank-1 matmul.
  - QKV projections for heads 2,3 and K/V are interleaved into pass-0's
    chunk loop so their PE work and DMA streaming overlap the attention;
    pass-0 Wo matmuls are interleaved into pass-1's loop.
  - All DRAM inputs are host-packed so every DMA is partition-major
    contiguous (KB-scale descriptor lines at full HBM rate).
  - SCALE is folded into Wq; RoPE's interleaved pairs are host-permuted to
    contiguous halves (cancels in q.k since both sides share it).
"""

import os
import sys
from contextlib import ExitStack

import numpy as np
import ml_dtypes

import concourse.bass as bass
import concourse.tile as tile
import concourse.mybir as mybir
from concourse import bacc
from concourse.bass_utils import run_bass_kernel_spmd
from concourse.masks import make_identity
from concourse.tile import add_dep_helper

BF16 = ml_dtypes.bfloat16

CORES = 8
B, S, D = 8, 32, 4096
H, KVH, HD = 32, 8, 128
SW = 4096
PREV = SW - S  # 4064
TOK = B * S  # 256
NH = H // KVH  # 4 Q heads per core
QCOLS = NH * HD  # 512
SCALE = float(HD) ** -0.5
N_DC = D // 128  # 32 contraction chunks
N_TC = 34  # t-chunks: 31 full cache + 96-tail + 2 new-token chunks
TAIL = PREV - 31 * 128  # 96
OUTC = D // CORES  # 512
NEG = -30.0  # additive bias for cross-batch new-token scores

# hd permutation: interleaved (r0,i0,r1,i1,...) -> (r..., i...)
_IDX = np.concatenate([np.arange(0, HD, 2), np.arange(1, HD, 2)])

LAST_EXEC_NS = None

_BUILD_CACHE = {}


def _install_ntff_hook():
    """Register the axon NTFF profiling hook (the agent image's antenv stub
    lacks axon_hooks). Only needed when tracing."""
    import types

    if "antenv.axon_hooks" in sys.modules:
        return
    try:
        from trn_agent_boot.trn_boot import _ntff_profile_via_ctypes

        hook = _ntff_profile_via_ctypes("/opt/axon/libaxon_pjrt.so")
    except Exception:
        hook = None
    mod = types.ModuleType("antenv.axon_hooks")
    mod._hook = hook
    mod.get_axon_ntff_profile_hook = lambda: mod._hook
    mod.set_axon_ntff_profile_hook = lambda h: setattr(mod, "_hook", h)
    sys.modules["antenv.axon_hooks"] = mod
    import antenv

    antenv.axon_hooks = mod


def build():
    dt = mybir.dt
    bf, f32 = dt.bfloat16, dt.float32
    EXP = mybir.ActivationFunctionType.Exp

    nc = bacc.Bacc("TRN2", target_bir_lowering=False, debug=False, num_devices=CORES)

    xt_d = nc.dram_tensor("xt", [128, N_DC, TOK], bf, kind="ExternalInput")
    wq_d = nc.dram_tensor("wq", [128, NH, N_DC, HD], bf, kind="ExternalInput")
    wkv_d = nc.dram_tensor("wkv", [128, N_DC, 2 * HD], bf, kind="ExternalInput")
    kct_d = nc.dram_tensor("kct", [HD, PREV], bf, kind="ExternalInput")
    vc_d = nc.dram_tensor("vc", [128, 32, HD], bf, kind="ExternalInput")
    wo_d = nc.dram_tensor("wo", [128, 2 * CORES * 2, OUTC], bf, kind="ExternalInput")
    cost_d = nc.dram_tensor("cost", [HD // 2, TOK], f32, kind="ExternalInput")
    sint_d = nc.dram_tensor("sint", [HD // 2, TOK], f32, kind="ExternalInput")
    biast_d = nc.dram_tensor("biast", [128, 2, TOK], f32, kind="ExternalInput")
    out_d = nc.dram_tensor("out", [128, 2, OUTC], bf, kind="ExternalOutput")

    with tile.TileContext(nc) as tc:
        with tc.tile_pool(name="const", bufs=1) as const:
            xt_sb = const.tile([128, N_DC, TOK], bf)
            wq_sb = const.tile([128, NH, N_DC, HD], bf)
            wkv_sb = const.tile([128, N_DC, 2 * HD], bf)
            kct_sb = const.tile([128, PREV], bf)
            vc_sb = const.tile([128, N_TC, HD], bf)
            wo_sb = const.tile([128, 2 * CORES * 2, OUTC], bf)
            cost_sb = const.tile([HD // 2, TOK], f32)
            sint_sb = const.tile([HD // 2, TOK], f32)
            biast_sb = const.tile([128, 2, TOK], f32)
            ones_col = const.tile([128, 1], bf)
            ones_row = const.tile([1, 128], f32)
            ident_sb = const.tile([128, 128], bf)
            qT_sb = [const.tile([128, 2, TOK], bf, tag=f"qT{p}") for p in range(2)]
            kTn_sb = const.tile([128, TOK], bf)
            vnT_sb = const.tile([128, TOK], bf)
            acc_sb = [const.tile([128, 2, TOK], f32, tag=f"acc{p}") for p in range(2)]
            rec_sb = [const.tile([1, 2 * TOK], f32, tag=f"rec{p}") for p in range(2)]
            recb_sb = [const.tile([128, 2 * TOK], f32, tag=f"recb{p}") for p in range(2)]
            all_sb = const.tile([128, 2, CORES, 2 * TOK], bf)
            out_sb = const.tile([128, 2, OUTC], bf)
            warm_rhs = const.tile([128, 512], bf)
            warm_sb = const.tile([1, 64], bf, name="warm_sb")

            # ---- on-device constants ----
            nc.vector.memset(ones_col[:], 1.0)
            nc.vector.memset(ones_row[:], 1.0)
            nc.vector.memset(warm_rhs[:], 0.0)
            make_identity(nc, ident_sb[:])
            nc.vector.memset(warm_sb[:], 0.0)

            # ---- input DMAs. sync queue carries the big weight stream in
            # first-use order (xt+wq01 interleaved, wq2, wq3, wkv, wo);
            # scalar queue carries the small early tensors + kct/vc.
            # Per-dma_start fixed cost dominates small pieces, and the 16
            # SDMA engines round-robin across ALL queued transfers at packet
            # granularity -- so everything issued up front completes together
            # at the smeared end. Tier the stream: only what the projections
            # need first is issued at t0 (split across the three DMA rings);
            # the rest is emitted behind dependency gates on q0's progress.
            nc.scalar.dma_start(out=cost_sb[:], in_=cost_d.ap())
            nc.scalar.dma_start(out=sint_sb[:], in_=sint_d.ap())
            nc.scalar.dma_start(out=biast_sb[:], in_=biast_d.ap())
            nc.scalar.dma_start(out=wq_sb[:, 0, 0:16, :], in_=wq_d.ap()[:, 0, 0:16, :])
            nc.scalar.dma_start(out=wq_sb[:, 0, 16:32, :], in_=wq_d.ap()[:, 0, 16:32, :])
            nc.scalar.dma_start(out=wq_sb[:, 1, 0:16, :], in_=wq_d.ap()[:, 1, 0:16, :])
            nc.scalar.dma_start(out=wq_sb[:, 1, 16:32, :], in_=wq_d.ap()[:, 1, 16:32, :])
            nc.sync.dma_start(out=xt_sb[:, 0:16, :], in_=xt_d.ap()[:, 0:16, :])
            nc.sync.dma_start(out=xt_sb[:, 16:32, :], in_=xt_d.ap()[:, 16:32, :])
            tier2 = [
                (nc.scalar, kct_sb[:], kct_d.ap()),
                (nc.scalar, vc_sb[:, 0:32, :], vc_d.ap()),
            ]
            tier3 = [
                (nc.sync, wq_sb[:, 2, :, :], wq_d.ap()[:, 2, :, :]),
                (nc.sync, wq_sb[:, 3, :, :], wq_d.ap()[:, 3, :, :]),
                (nc.sync, wkv_sb[:], wkv_d.ap()),
                (nc.sync, wo_sb[:, 0:16, :], wo_d.ap()[:, 0:16, :]),
                (nc.sync, wo_sb[:, 16:32, :], wo_d.ap()[:, 16:32, :]),
            ]

            with tc.tile_pool(name="rope_tmp", bufs=4) as rtmp:

                def rope(src_ps, dst):
                    hh = HD // 2
                    qr, qi = src_ps[0:hh, :], src_ps[hh:128, :]
                    t1 = rtmp.tile([hh, TOK], f32, tag="t1")
                    t2 = rtmp.tile([hh, TOK], f32, tag="t2")
                    nc.vector.tensor_mul(t1[:], qr, cost_sb[:])
                    nc.vector.tensor_mul(t2[:], qi, sint_sb[:])
                    nc.vector.tensor_sub(dst[0:hh, :], t1[:], t2[:])
                    t3 = rtmp.tile([hh, TOK], f32, tag="t1")
                    t4 = rtmp.tile([hh, TOK], f32, tag="t2")
                    nc.vector.tensor_mul(t3[:], qr, sint_sb[:])
                    nc.vector.tensor_mul(t4[:], qi, cost_sb[:])
                    nc.vector.tensor_add(dst[hh:128, :], t3[:], t4[:])

                # ---- PE warmup: ~4us of matmuls so the HAM clock gate
                # reaches full rate as the first projections arrive ----
                with tc.tile_pool(name="warm_ps", bufs=1, space="PSUM") as wp:
                    wps = wp.tile([128, 512], f32)
                    for _ in range(22):
                        nc.tensor.matmul(
                            wps[:], warm_rhs[:, 0:128], warm_rhs[:],
                            start=True, stop=True, skip_group_check=True,
                        )

                # ---- collective plumbing: HBM bounce buffers + a tiny
                # warmup AllGather so the first real gather's trigger->start
                # latency is small ----
                dram = ctx_dram = tc.tile_pool(name="dram", bufs=1, space="DRAM")
                dram = ctx_dram.__enter__()
                ag_in = [dram.tile([128, 2 * TOK], bf, tag=f"agi{p}", name=f"agi{p}") for p in range(2)]
                ag_out = [
                    dram.tile([128 * CORES, 2 * TOK], bf, tag=f"ago{p}", name=f"ago{p}",
                              addr_space="Shared")
                    for p in range(2)
                ]
                # warmup AllGather: fires the CC doorbell as gpsimd's first
                # instruction (~8us) -- the first collective's internal
                # startup is 45-108us, and on slow runs it otherwise blocks
                # AG-0. Input content is irrelevant, so no bounce DMA.
                agw_in = dram.tile([1, 64], bf, name="agw_in")
                agw_out = dram.tile([CORES, 64], bf, name="agw_out", addr_space="Shared")
                nc.gpsimd.collective_compute(
                    "AllGather", mybir.AluOpType.bypass,
                    replica_groups=[list(range(CORES))],
                    ins=[agw_in.opt()], outs=[agw_out.opt()],
                )

                # long-lived attention pools (both passes)
                with ExitStack() as stk:
                    s_pool = stk.enter_context(tc.tile_pool(name="s_ps", bufs=3, space="PSUM"))
                    o_pool = stk.enter_context(tc.tile_pool(name="o_ps", bufs=1, space="PSUM"))
                    misc_pool = stk.enter_context(tc.tile_pool(name="misc_ps", bufs=1, space="PSUM"))
                    attn_pool = stk.enter_context(tc.tile_pool(name="attn", bufs=5))
                    proj_cm = tc.tile_pool(name="proj_ps", bufs=2, space="PSUM")
                    proj_pool = proj_cm.__enter__()

                    def proj(dst, wsrc, c0, c1, nchunk):
                        for c in range(c0, c1):
                            nc.tensor.matmul(
                                dst, wsrc[:, c, :], xt_sb[:, c, :],
                                start=(c == 0), stop=(c == nchunk - 1),
                                skip_group_check=True,
                            )

                    # q0/q1 ahead of the pass-0 loop
                    q01 = {}
                    for h in (0, 1):
                        q01[h] = proj_pool.tile([128, TOK], f32, tag="pj")
                        proj(q01[h][:], wq_sb[:, h], 0, N_DC, N_DC)
                        rope(q01[h][:], qT_sb[0][:, h, :])

                    # deferred-projection schedule, interleaved into pass-0:
                    # chunk idx -> emit fn
                    q23 = {}
                    kv = {}

                    def mk_qproj(h, c0, c1):
                        def f():
                            if h not in q23:
                                q23[h] = proj_pool.tile([128, TOK], f32, tag="pj")
                            proj(q23[h][:], wq_sb[:, h], c0, c1, N_DC)
                        return f

                    def mk_kvproj(which, off, c0, c1):
                        def f():
                            if which not in kv:
                                kv[which] = proj_pool.tile([128, TOK], f32, tag="pj")
                            for c in range(c0, c1):
                                nc.tensor.matmul(
                                    kv[which][:], wkv_sb[:, c, off : off + HD],
                                    xt_sb[:, c, :],
                                    start=(c == 0), stop=(c == N_DC - 1),
                                    skip_group_check=True,
                                )
                        return f

                    def vn_emit():
                        # V_new^T -> V_new chunks [tok, hd] via PE transpose
                        nc.scalar.copy(vnT_sb[:], kv["v"][:])
                        for j in range(2):
                            vt = misc_pool.tile([128, 128], bf, tag="vt")
                            nc.tensor.transpose(
                                vt[:], vnT_sb[:, j * 128 : (j + 1) * 128],
                                ident_sb[:],
                            )
                            nc.scalar.copy(vc_sb[:, 32 + j, :], vt[:])

                    sched = {
                        2: mk_qproj(2, 0, 11), 3: mk_qproj(2, 11, 22),
                        4: mk_qproj(2, 22, N_DC),
                        5: lambda: rope(q23[2][:], qT_sb[1][:, 0, :]),
                        6: mk_qproj(3, 0, 11), 7: mk_qproj(3, 11, 22),
                        8: mk_qproj(3, 22, N_DC),
                        9: lambda: rope(q23[3][:], qT_sb[1][:, 1, :]),
                        12: mk_kvproj("v", HD, 0, 11),
                        13: mk_kvproj("v", HD, 11, 22),
                        14: mk_kvproj("v", HD, 22, N_DC),
                        15: vn_emit,
                        18: mk_kvproj("k", 0, 0, 11),
                        19: mk_kvproj("k", 0, 11, 22),
                        20: mk_kvproj("k", 0, 22, N_DC),
                        21: lambda: rope(kv["k"][:], kTn_sb[:]),
                    }

                    def pass_loop(p, extra_sched, order=None):
                        qpair = qT_sb[p][:, :, :]
                        o_ps = o_pool.tile([128, 2, TOK], f32, tag="o")
                        prev = None
                        for ti, t in enumerate(order):
                            n = 128 if t != 31 else TAIL
                            if t < 32:
                                lhs = kct_sb[:, t * 128 : t * 128 + n]
                            else:
                                lhs = kTn_sb[:, (t - 32) * 128 : (t - 31) * 128]
                            s_ps = s_pool.tile([128, 2, TOK], f32, tag="s")
                            nc.tensor.matmul(
                                s_ps[0:n, :, :], lhs, qpair,
                                start=True, stop=True, skip_group_check=True,
                            )
                            for f in (extra_sched.get(ti) or []):
                                f()
                            if len(pending) == 3:
                                sum_av(*pending.pop(0))
                            if t >= 32:
                                nc.vector.scalar_tensor_tensor(
                                    out=s_ps[:, :, :],
                                    in0=s_ps[:, :, :],
                                    scalar=0.0,
                                    in1=biast_sb[:, t - 32, :]
                                    .unsqueeze(1)
                                    .broadcast_to((128, 2, TOK)),
                                    op0=mybir.AluOpType.add,
                                    op1=mybir.AluOpType.add,
                                )
                            a = attn_pool.tile([128, 2, TOK], bf, tag="a")
                            nc.scalar.activation(a[0:n, :, :], s_ps[0:n, :, :], EXP)
                            nc.vector.tensor_add(
                                acc_sb[p][0:n, :, :], acc_sb[p][0:n, :, :],
                                a[0:n, :, :],
                            )
                            prev = (a, n, t)
                        pa, pn, pt = prev
                        nc.tensor.matmul(
                            o_ps[:, :, :], vc_sb[0:pn, pt, :], pa[0:pn, :, :],
                            start=False, stop=True, skip_group_check=True,
                        )
                        # rowsum reduce + reciprocal + partition-broadcast,
                        # all through one PSUM bank (sum lives in row 0 until
                        # the broadcast matmul reclaims the bank)
                        sr_ps = misc_pool.tile([128, 2 * TOK], f32, tag="sr")
                        nc.tensor.matmul(
                            sr_ps[0:1, :], ones_col[:, 0:1],
                            acc_sb[p].rearrange("p h s -> p (h s)"),
                            start=True, stop=True, skip_group_check=True,
                        )
                        nc.vector.reciprocal_approx_fast(rec_sb[p][:], sr_ps[0:1, :])
                        nc.tensor.matmul(
                            sr_ps[:, :], ones_row[0:1, :], rec_sb[p][:],
                            start=True, stop=True, skip_group_check=True,
                        )
                        nc.scalar.copy(recb_sb[p][:], sr_ps[:, :])
                        nc.vector.tensor_mul(
                            all_sb[:, p, 0, :],
                            o_ps[:, :, :].rearrange("p h s -> p (h s)"),
                            recb_sb[p][:],
                        )
                        nc.scalar.dma_start(ag_in[p][:], all_sb[:, p, 0, :])
                        nc.gpsimd.collective_compute(
                            "AllGather", mybir.AluOpType.bypass,
                            replica_groups=[list(range(CORES))],
                            ins=[ag_in[p].opt()], outs=[ag_out[p].opt()],
                        )
                        ag_r = ag_out[p].rearrange("(r q) n -> q r n", q=128)
                        nc.sync.dma_start(all_sb[:, p, 0:4, :], ag_r[:, 0:4, :])
                        nc.sync.dma_start(all_sb[:, p, 4:8, :], ag_r[:, 4:8, :])

                    # ---- pass 0 (heads 0,1) with projections interleaved ----
                    pass_loop(0, {t: [f] for t, f in sched.items()})
                    proj_cm.__exit__(None, None, None)

                    # ---- pass 1 (heads 2,3) with pass-0 Wo interleaved ----
                    with tc.tile_pool(name="wo_ps", bufs=1, space="PSUM") as wo_pool:
                        out_ps = [
                            wo_pool.tile([128, OUTC], f32, tag=f"out{k}")
                            for k in range(2)
                        ]

                        def wo_block(p, s, l, first, last):
                            g = p * CORES * 2 + s * 2 + l
                            for k in range(2):
                                nc.tensor.matmul(
                                    out_ps[k],
                                    all_sb[:, p, s, l * TOK + k * 128 : l * TOK + (k + 1) * 128],
                                    wo_sb[:, g, :],
                                    start=first, stop=last,
                                    skip_group_check=True,
                                )

                        pass_loop(1, {}, order=[32, 33] + list(range(32)))

                        for s in range(CORES):
                            for l in range(2):
                                wo_block(0, s, l, s == 0 and l == 0, False)
                        # final pass k-major: finish (and ship) the first
                        # token-half while the second is still accumulating
                        for k in range(2):
                            for s in range(CORES):
                                for l in range(2):
                                    nc.tensor.matmul(
                                        out_ps[k],
                                        all_sb[:, 1, s, l * TOK + k * 128 : l * TOK + (k + 1) * 128],
                                        wo_sb[:, CORES * 2 + s * 2 + l, :],
                                        start=False,
                                        stop=(s == CORES - 1 and l == 1),
                                        skip_group_check=True,
                                    )
                            nc.scalar.copy(out_sb[:, k, :], out_ps[k])
                            nc.sync.dma_start(out_d.ap()[:, k, :], out_sb[:, k, :])
                ctx_dram.__exit__(None, None, None)

    nc.compile()
    return nc


def prep_in_maps(x, freqs_cos, freqs_sin, mask, cache_k, cache_v, Wq, Wk, Wv, Wo,
                 slotmap):
    x = np.asarray(x, np.float32).reshape(TOK, D)
    xt = x.T.reshape(N_DC, 128, TOK).transpose(1, 0, 2)  # [128, c, tok]
    cost = np.ascontiguousarray(
        np.tile(np.asarray(freqs_cos, np.float32)[0].T, (1, B))
    )
    sint = np.ascontiguousarray(
        np.tile(np.asarray(freqs_sin, np.float32)[0].T, (1, B))
    )
    mask = np.asarray(mask, np.float32)  # [B, S, S] additive
    # biast[r, ci, n]: additive bias for new-token row j = ci*128 + r
    # (batch j//32, pos j%32) against query token n (batch n//32, pos n%32)
    j = np.arange(2 * 128)
    n = np.arange(TOK)
    same = (j[:, None] // S) == (n[None, :] // S)
    biast = np.where(same, mask[j[:, None] // S, n[None, :] % S, j[:, None] % S], NEG)
    biast = biast.reshape(2, 128, TOK).transpose(1, 0, 2)  # [128, ci, tok]

    Wq = np.asarray(Wq, np.float32)
    Wk = np.asarray(Wk, np.float32)
    Wv = np.asarray(Wv, np.float32)
    Wo = np.asarray(Wo, np.float32)
    cache_k = np.asarray(cache_k, np.float32)
    cache_v = np.asarray(cache_v, np.float32)

    in_maps = []
    for c in range(CORES):
        wq_c = (Wq[:, c * QCOLS : (c + 1) * QCOLS] * SCALE).reshape(D, NH, HD)[
            :, :, _IDX
        ]  # [D, h, hd]
        wq_c = wq_c.reshape(N_DC, 128, NH, HD).transpose(1, 2, 0, 3)  # [128,h,c,hd]
        wk_c = Wk[:, c * HD : (c + 1) * HD][:, _IDX]
        wv_c = Wv[:, c * HD : (c + 1) * HD]
        wkv_c = np.concatenate([wk_c, wv_c], axis=1)  # [D, 256]
        wkv_c = wkv_c.reshape(N_DC, 128, 2 * HD).transpose(1, 0, 2)
        kct_c = np.ascontiguousarray(cache_k[0, :PREV, c, :][:, _IDX].T)
        vc_c = np.zeros((32, 128, HD), np.float32)
        vc_c.reshape(32 * 128, HD)[:PREV] = cache_v[0, :PREV, c, :]
        vc_c = vc_c.transpose(1, 0, 2)  # [128, c, hd]
        # Wo row-blocks in (pass, slot, l) order using the probed slot map
        wo_c = np.empty((2, CORES, 2, HD, OUTC), np.float32)
        for p in range(2):
            for s in range(CORES):
                g = slotmap[c][s]
                for l in range(2):
                    h = g * NH + p * 2 + l
                    wo_c[p, s, l] = Wo[h * HD : (h + 1) * HD, c * OUTC : (c + 1) * OUTC]
        wo_c = wo_c.reshape(32, HD, OUTC).transpose(1, 0, 2)  # [128, g, outc]
        in_maps.append(
            {
                "xt": np.ascontiguousarray(xt).astype(BF16),
                "wq": np.ascontiguousarray(wq_c).astype(BF16),
                "wkv": np.ascontiguousarray(wkv_c).astype(BF16),
                "kct": kct_c.astype(BF16),
                "vc": np.ascontiguousarray(vc_c).astype(BF16),
                "wo": np.ascontiguousarray(wo_c).astype(BF16),
                "cost": cost,
                "sint": sint,
                "biast": np.ascontiguousarray(biast).astype(np.float32),
            }
        )
    return in_maps


def kernel(x, freqs_cos, freqs_sin, mask, cache_k, cache_v, Wq, Wk, Wv, Wo, positions):
    global LAST_EXEC_NS
    assert int(positions) == PREV, f"kernel compiled for positions={PREV}"

    slotmap = [list(range(CORES))] * CORES  # AllGather output is rank-ordered

    if "main" not in _BUILD_CACHE:
        _BUILD_CACHE["main"] = build()
    nc = _BUILD_CACHE["main"]

    in_maps = prep_in_maps(
        x, freqs_cos, freqs_sin, mask, cache_k, cache_v, Wq, Wk, Wv, Wo, slotmap
    )

    trace = os.environ.get("KERNEL_TRACE", "0") == "1"
    if trace:
        _install_ntff_hook()
    res = run_bass_kernel_spmd(
        nc, in_maps, core_ids=list(range(CORES)), trace=trace
    )
    if trace:
        LAST_EXEC_NS = res.exec_time_ns

    out = np.empty((TOK, D), np.float32)
    for c in range(CORES):
        o = np.asarray(res.results[c]["out"], np.float32)  # [128, 2, OUTC]
        out[:, c * OUTC : (c + 1) * OUTC] = o.transpose(1, 0, 2).reshape(TOK, OUTC)
    return out.reshape(B, S, D)
